# revision 8
# baseline (speedup 1.0000x reference)
"""Trainium2 Bass kernel for nn_Attention_36404142801494.

Fused causal self-attention (q=k=v=Wq(x)) + output projection, sharded over
8 NeuronCores: data-parallel on batch (B=2 -> 2 groups of 4 cores), tensor-
parallel on heads (8 heads -> 2 heads/core) with a column-split Wq and a
row-split Wo. Each core returns a partial [S, HID] output; the host sums the
4 partials per batch and adds the Wo bias while unsharding.

v2 design (vs the 77.7us baseline):
  - qT kept in bf16 so every attention matmul runs at 1 cycle/row and the
    causal diagonal band can be trimmed to sub-256-col matmuls.
  - Diagonal trimming everywhere: QK matmul cols, exp cols, AV cols. The
    per-chunk causal mask is a single [128,128] upper-tri multiply.
  - exp groups: full chunks in [128,1024] pairs; the 4 diagonal chunks
    packed [512|384] + [256|128] to minimize ACT columns + instr count.
  - Softmax denominator from a 65th all-ones lhsT column in the AV matmul;
    normalization = DVE reciprocal of the PSUM den row -> gpsimd
    partition_broadcast -> one DVE multiply that doubles as the PSUM->SBUF
    move into the bf16 ao tile (no DRAM bounce).
  - Wo: both heads' ao stacked [128, S] -> ONE matmul per 128-token block.
  - QK/exp/AV software-pipelined per 2-chunk group; qproj / V transposes /
    Wo blocks emitted between attention units as PE filler.
Everything hardcoded for B=2, S=2048, HID=512, NH=8, HD=64.
"""

import sys

sys.path.insert(0, "/opt/trn_rl_repo")

import numpy as np
import ml_dtypes

import concourse.bass as bass
import concourse.bacc as bacc
import concourse.tile as tile
import concourse.mybir as mybir
from concourse.bass_utils import run_bass_kernel_spmd
from concourse.masks import make_identity

f32 = mybir.dt.float32
f32r = mybir.dt.float32r
bf16 = mybir.dt.bfloat16
EXPT_DT = bf16

B, S, HID = 2, 2048, 512
NH, HD = 8, 64
N_CORES = 8
SB = 512           # query-block width
NSB = S // SB      # 4 query blocks
SCALE = 1.0 / np.sqrt(HD)

Exp = mybir.ActivationFunctionType.Exp


def build_nc():
    nc = bacc.Bacc(None, target_bir_lowering=False)

    xP = nc.dram_tensor("xP", [128, 4, S], bf16, kind="ExternalInput")
    WqP = nc.dram_tensor("WqP", [128, 4 * 128], bf16, kind="ExternalInput")
    Wqb = nc.dram_tensor("Wqb", [128, 1], f32, kind="ExternalInput")
    WoP = nc.dram_tensor("WoP", [128, HID], bf16, kind="ExternalInput")
    TriM = nc.dram_tensor("TriM", [128, 128], bf16, kind="ExternalInput")
    out_part = nc.dram_tensor("out_part", [S, HID], f32, kind="ExternalOutput")

    with tile.TileContext(nc) as tc:
        with (
            tc.tile_pool(name="singles", bufs=1) as singles,
            tc.tile_pool(name="etp", bufs=6) as etp,
            tc.tile_pool(name="rrp", bufs=3) as rrp,
            tc.tile_pool(name="bcp", bufs=3) as bcp,
            tc.tile_pool(name="obp", bufs=4) as obp,
        ):
            # ---- input loads. critical path: wq, x0 (split per hid-chunk so
            # qproj's accumulation chases the DMA), wqb; x1-x3 via the scalar
            # HWDGE queue so the sync queue stays clear for out-writes.
            wq = singles.tile([128, 4, 128], bf16, tag="wq")
            nc.sync.dma_start(out=wq[:, :, :], in_=WqP[:, :])
            xs = singles.tile([128, 4, S], bf16, tag="xs")
            nc.sync.dma_start(out=xs[:, 0, 0:SB], in_=xP[:, 0, 0:SB])
            wqb = singles.tile([128, 1], f32, tag="wqb")
            nc.sync.dma_start(out=wqb, in_=Wqb[:, :])
            for i in range(1, 4):
                nc.sync.dma_start(out=xs[:, i, 0:SB], in_=xP[:, i, 0:SB])
            for sb in range(1, NSB):
                s0 = sb * SB
                nc.scalar.dma_start(
                    out=xs[:, :, s0 : s0 + SB], in_=xP[:, :, s0 : s0 + SB]
                )
            # non-critical loads via the gpsimd (SWDGE) queue
            wo = singles.tile([128, HID], bf16, tag="wo")
            nc.gpsimd.dma_start(out=wo, in_=WoP[:, :])
            tri = singles.tile([128, 128], bf16, tag="tri")
            nc.gpsimd.dma_start(out=tri, in_=TriM[:, :])

            # ---- constants ----
            identf = singles.tile([128, 64], f32, tag="identf")
            make_identity(nc, identf[0:64, :])
            nc.gpsimd.memset(identf[64:128, :], 0.0)
            nc.gpsimd.affine_select(
                out=identf[64:128, :], in_=identf[64:128, :],
                compare_op=mybir.AluOpType.not_equal,
                fill=1.0, base=0, pattern=[[-1, 64]], channel_multiplier=1,
            )
            ident = singles.tile([128, 64], bf16, tag="ident")
            nc.vector.tensor_copy(ident, identf)

            # preload the exp ACT table while DMAs stream in
            preld = singles.tile([32, 32], f32, tag="preld")
            nc.vector.memset(preld, 0.0)
            nc.scalar.activation(out=preld, in_=preld, func=Exp, scale=1.0)

            qT = singles.tile([128, S], bf16, tag="qT")
            ao = singles.tile([128, S], bf16, tag="ao")
            v_sb = [
                singles.tile([128, 16, 65], bf16, name=f"v{h}", tag=f"v{h}")
                for h in range(2)
            ]
            for h in range(2):
                nc.gpsimd.memset(v_sb[h][:, :, 64:65], 1.0)

            with (
                tc.tile_pool(name="qpps", bufs=2, space="PSUM") as qpps,
                tc.tile_pool(name="qkps", bufs=2, space="PSUM") as qkps,
                tc.tile_pool(name="avps", bufs=2, space="PSUM") as avps,
            ):
                def qproj(sb):
                    s0 = sb * SB
                    qp = qpps.tile([128, SB], f32, tag="ps1", name="qp")
                    for i in range(4):
                        nc.tensor.matmul(
                            qp, lhsT=wq[:, i, :], rhs=xs[:, i, s0 : s0 + SB],
                            start=(i == 0), stop=(i == 3),
                        )
                    nc.vector.tensor_scalar_add(qT[:, s0 : s0 + SB], qp, wqb)

                def vprep(h, tq):
                    hp = 64 * h
                    vt = qpps.tile([128, 4, 64], bf16, tag="ps1", name="vt")
                    for j in range(4):
                        t0 = 128 * (4 * tq + j)
                        nc.tensor.transpose(
                            vt[:, j, :], qT[hp : hp + 64, t0 : t0 + 128],
                            ident[hp : hp + 64, :],
                        )
                    nc.vector.tensor_copy(v_sb[h][:, 4 * tq : 4 * tq + 4, 0:64], vt)

                def attn_unit(h, sb):
                    """Returns the (live) av PSUM tile [65, SB] for this unit."""
                    hp = 64 * h
                    s0 = sb * SB
                    nch = 4 * sb + 4
                    d0 = 4 * sb
                    # groups of (chunk, tile col offset, width, query col offset)
                    groups = [
                        [(c, 0, SB, 0), (c + 1, SB, SB, 0)]
                        for c in range(0, 4 * sb, 2)
                    ]
                    groups.append([(d0, 0, 512, 0), (d0 + 1, 512, 384, 128)])
                    groups.append([(d0 + 2, 0, 256, 256), (d0 + 3, 256, 128, 384)])

                    av = avps.tile([65, SB], f32, tag="av", name="av")

                    def emit_av(et, spec):
                        for ci, off, w, qoff in spec:
                            nc.tensor.matmul(
                                av[:, qoff : qoff + w],
                                lhsT=v_sb[h][:, ci, :],
                                rhs=et[:, off : off + w],
                                start=(ci == 0), stop=(ci == nch - 1),
                                skip_group_check=True,
                            )

                    prev = None
                    for spec in groups:
                        qk = qkps.tile([128, 2 * SB], f32, tag="qk", name="qk")
                        wtot = spec[-1][1] + spec[-1][2]
                        for ci, off, w, qoff in spec:
                            nc.tensor.matmul(
                                qk[:, off : off + w],
                                lhsT=qT[hp : hp + 64, 128 * ci : 128 * ci + 128],
                                rhs=qT[hp : hp + 64, s0 + qoff : s0 + SB],
                                start=True, stop=True,
                            )
                        if prev is not None:
                            emit_av(*prev)
                        et = etp.tile([128, 2 * SB], EXPT_DT, tag="et", name="et")
                        nc.scalar.activation(
                            out=et[:, 0:wtot], in_=qk[:, 0:wtot], func=Exp, scale=SCALE
                        )
                        for ci, off, w, qoff in spec:
                            if ci >= d0:
                                nc.vector.tensor_mul(
                                    et[:, off : off + 128], et[:, off : off + 128], tri
                                )
                        prev = (et, spec)
                    emit_av(*prev)
                    return av

                def norm(h, sb, av, c0=0, cw=SB):
                    hp = 64 * h
                    s0 = sb * SB
                    rrow = rrp.tile([1, SB], f32, name="rrow")
                    nc.vector.reciprocal(rrow[:, 0:cw], av[64:65, c0 : c0 + cw])
                    bct = bcp.tile([64, SB], f32, name="bct")
                    nc.gpsimd.partition_broadcast(bct[:, 0:cw], rrow[0:1, 0:cw])
                    nc.vector.tensor_mul(
                        ao[hp : hp + 64, s0 + c0 : s0 + c0 + cw],
                        av[0:64, c0 : c0 + cw],
                        bct[:, 0:cw],
                    )

                def wo_sc(sc):
                    c0 = 128 * sc
                    wp = qpps.tile([128, HID], f32, tag="ps1", name="wp")
                    nc.tensor.matmul(
                        wp, lhsT=ao[:, c0 : c0 + 128], rhs=wo,
                        start=True, stop=True,
                    )
                    ob = obp.tile([128, HID], f32, tag="ob", name="ob")
                    nc.vector.tensor_copy(ob, wp)
                    nc.sync.dma_start(out=out_part[c0 : c0 + 128, :], in_=ob)

                def wo_block(sb):
                    for sc in range(4 * sb, 4 * sb + 4):
                        wo_sc(sc)

                qproj(0)
                vprep(0, 0)
                vprep(1, 0)
                for sb in range(NSB):
                    av0 = attn_unit(0, sb)
                    if sb < 3:
                        qproj(sb + 1)
                    norm(0, sb, av0)
                    if sb >= 1:
                        wo_block(sb - 1)
                    if sb < 3:
                        vprep(0, sb + 1)
                        vprep(1, sb + 1)
                    av1 = attn_unit(1, sb)
                    if sb < 3:
                        norm(1, sb, av1)
                # tail: pipeline h1/sb=3's normalization + Wo at 128-query
                # granularity so out-DMAs start as early as possible
                for j in range(4):
                    norm(1, 3, av1, c0=128 * j, cw=128)
                    wo_sc(12 + j)

    nc.finalize()
    return nc


_NC_CACHE = None


def _get_nc():
    global _NC_CACHE
    if _NC_CACHE is None:
        _NC_CACHE = build_nc()
    return _NC_CACHE


def make_in_maps(x, Wq_w, Wq_b, Wo_w):
    x = np.asarray(x, dtype=np.float32)
    Wq_w = np.asarray(Wq_w, dtype=np.float32)
    Wq_b = np.asarray(Wq_b, dtype=np.float32)
    Wo_w = np.asarray(Wo_w, dtype=np.float32)
    tri = np.triu(np.ones((128, 128), dtype=np.float32)).astype(ml_dtypes.bfloat16)
    in_maps = []
    for c in range(N_CORES):
        b, hp = divmod(c, 4)
        dq = slice(128 * hp, 128 * (hp + 1))
        # xP[p, i, s] = x[b].T[128i + p, s]
        xT = np.ascontiguousarray(x[b].T)                      # [512, 2048]
        xPk = np.ascontiguousarray(xT.reshape(4, 128, S).transpose(1, 0, 2))
        # WqP[k, 128i + m] = Wq_w[dq, :].T[128i + k, m]
        WqT = Wq_w[dq, :].T                                    # [512, 128]
        WqPk = np.ascontiguousarray(
            WqT.reshape(4, 128, 128).transpose(1, 0, 2).reshape(128, 512)
        )
        in_maps.append({
            "xP": xPk.astype(ml_dtypes.bfloat16),
            "WqP": WqPk.astype(ml_dtypes.bfloat16),
            "Wqb": np.ascontiguousarray(Wq_b[dq].reshape(128, 1)),
            "WoP": np.ascontiguousarray(Wo_w[:, dq].T).astype(ml_dtypes.bfloat16),
            "TriM": tri,
        })
    return in_maps


def kernel(x, mask, Wq_w, Wq_b, Wo_w, Wo_b, **_):
    nc = _get_nc()
    in_maps = make_in_maps(x, Wq_w, Wq_b, Wo_w)
    res = run_bass_kernel_spmd(nc, in_maps, core_ids=list(range(N_CORES)))
    Wo_b = np.asarray(Wo_b, dtype=np.float32)
    out = np.empty((B, S, HID), dtype=np.float32)
    for b in range(B):
        acc = res.results[4 * b]["out_part"].astype(np.float32)
        for c in range(4 * b + 1, 4 * b + 4):
            acc = acc + res.results[c]["out_part"]
        out[b] = acc + Wo_b[None, :]
    return out
